# revision 37
# baseline (speedup 1.0000x reference)
"""Trainium2 Bass kernel for nn_DynamicEarlyExitStudent.

3-layer Mamba-style selective-SSM student network:
  tokenize (embed lookups + linear fusion + LN) -> 3x mamba -> LN -> heads.

Sharding: data-parallel over batch B=16 across 8 cores (2 rows/core),
params replicated, no collectives. Each core computes its rows end-to-end.

Device layout: activations are kept transposed [channels, tokens] with
channels on SBUF partitions; tokens t = b*512 + l (2 local batch rows).
The L-sequential scan runs on the Vector engine via tensor_tensor_scan
(state = dA*state + dBu along the free dim), vectorized over 128 channels
per instruction, one instruction per (state-index n, d-tile, batch row).
Embedding lookups are folded into threshold-comparison features times a
host-precomposed matrix; the n-reduction of h*C runs on the TensorEngine
as identity-matmul PSUM accumulation. Matmuls use float32r (full-rate).
"""

import sys

for _p in ("/opt/trn_rl_repo",):
    if _p not in sys.path:
        sys.path.insert(0, _p)

from contextlib import ExitStack

import numpy as np

import concourse.bacc as bacc
import concourse.bass as bass
import concourse.tile as tile
from concourse import masks, mybir
from concourse.bass_utils import run_bass_kernel_spmd

# Model dims
D, DI, N, R, KC, NL = 256, 512, 16, 16, 4, 3
B, L = 16, 512
NCORES = 8
BL = B // NCORES          # local batch rows per core
T = BL * L                # local tokens
P = 128                   # SBUF partitions
F32 = mybir.dt.float32
F32R = mybir.dt.float32r
BF16 = mybir.dt.bfloat16
AF = mybir.ActivationFunctionType
OP = mybir.AluOpType

NKT_D = D // P            # 2 k-tiles over D
NDT = DI // P             # 4 d-tiles over DI
NNH = T // 512            # matmul N-halves (fp32 moving max 512)

# tokenizer comparison-feature k-tiles (rows per tile). Tile 2 layout:
# [0:63]=flag thresholds 1..63, 63=unused(zero), 64=dir, 65=len, 66=iat,
# 67=const  (engine writes may only start at partitions 0/32/64/96).
TOK_K = (128, 127, 68)


def r32(ap):
    return ap.bitcast(F32R)


def host_prep(inputs):
    """Precompute combined weight matrices on host (small, param-only)."""
    f = lambda k: np.asarray(inputs[k], np.float32)
    emb_proto, emb_flags, emb_dir = f("emb_proto"), f("emb_flags"), f("emb_dir")
    fus_w, fus_b = f("fus_w"), f("fus_b")
    len_w, len_b, iat_w, iat_b = f("len_w"), f("len_b"), f("iat_w"), f("iat_b")
    Wp, Wf, Wd = fus_w[:, 0:32], fus_w[:, 32:64], fus_w[:, 64:72]
    Wl, Wi = fus_w[:, 72:104], fus_w[:, 104:136]

    # one-hot(trunc(x)) @ E == const_row + sum_v (x>=v) * (E[v]-E[v-1])
    dproto = (emb_proto[1:] - emb_proto[:-1]) @ Wp.T          # [255, D]
    dflags = (emb_flags[1:] - emb_flags[:-1]) @ Wf.T          # [63, D]
    ddir = (emb_dir[1] - emb_dir[0]) @ Wd.T                   # [D]
    row_len = len_w[:, 0] @ Wl.T                              # [D]
    row_iat = iat_w[:, 0] @ Wi.T                              # [D]
    row_const = (
        emb_proto[0] @ Wp.T + emb_flags[0] @ Wf.T + emb_dir[0] @ Wd.T
        + len_b @ Wl.T + iat_b @ Wi.T + fus_b
    )
    wtok = np.zeros((sum(TOK_K), D), np.float32)
    wtok[0:255] = dproto
    wtok[255:318] = dflags
    wtok[319] = ddir
    wtok[320] = row_len
    wtok[321] = row_iat
    wtok[322] = row_const

    prep = {
        "wtok": wtok,
        "inw": np.ascontiguousarray(f("in_w").transpose(0, 2, 1)),    # [NL,256,1024]
        "xpw": np.ascontiguousarray(f("xp_w").transpose(0, 2, 1)),    # [NL,512,48]
        "dtw": np.ascontiguousarray(f("dt_w").transpose(0, 2, 1)),    # [NL,16,512]
        "outw": np.ascontiguousarray(f("out_w").transpose(0, 2, 1)),  # [NL,512,256]
        "convw": np.ascontiguousarray(f("conv_w")[:, :, 0, :]),       # [NL,512,4]
        "convb": f("conv_b"),
        "dtb": f("dt_b"),
        "amat": np.ascontiguousarray(-np.exp(f("A_log"))),            # [NL,512,16]
        "dsk": f("D_skip"),
        "tokg": f("tok_g"), "tokb": f("tok_b"),
        "normg": f("norm_g"), "normb": f("norm_b"),
        "headw": np.ascontiguousarray(
            np.concatenate([f("cls_w"), f("halt_w")], 0).T),          # [256, 3]
        "headb": np.concatenate([f("cls_b"), f("halt_b")], 0),        # [3]
    }
    return prep


# (shape, is_matmul_operand) - matmul operands are float32r end to end
PARAM_SPECS = {
    "wtok": ((sum(TOK_K), D), True),
    "inw": ((NL, D, 2 * DI), True),
    "xpw": ((NL, DI, R + 2 * N), True),
    "dtw": ((NL, R, DI), True),
    "outw": ((NL, DI, D), True),
    "convw": ((NL, DI, KC), False),
    "convb": ((NL, DI), False),
    "dtb": ((NL, DI), False),
    "amat": ((NL, DI, N), False),
    "dsk": ((NL, DI), False),
    "tokg": ((D,), False), "tokb": ((D,), False),
    "normg": ((D,), False), "normb": ((D,), False),
    "headw": ((D, 3), True),
    "headb": ((3,), False),
}


def build_program():
    nc = bacc.Bacc("TRN2", target_bir_lowering=False, debug=False)

    x5 = nc.dram_tensor("x5", [5, T], F32R, kind="ExternalInput").ap()
    dr = {
        k: nc.dram_tensor(k, list(shp), F32R if isr else F32,
                          kind="ExternalInput").ap()
        for k, (shp, isr) in PARAM_SPECS.items()
    }
    out3_dram = nc.dram_tensor("out3", [3, T], F32, kind="ExternalOutput").ap()

    with tile.TileContext(nc) as tc, ExitStack() as ctx:
        wpool = ctx.enter_context(tc.tile_pool(name="wpool", bufs=1))
        dpool = ctx.enter_context(tc.tile_pool(name="dpool", bufs=2,
                                               space="DRAM"))
        apool = ctx.enter_context(tc.tile_pool(name="apool", bufs=1))
        spool = ctx.enter_context(tc.tile_pool(name="spool", bufs=3))

        # ---- constants (f32r tiles cannot be memset directly; build in
        # f32 and convert via ScalarE copy, which rounds to f32r on write)
        ones_f = wpool.tile([P, P], F32)
        nc.vector.memset(ones_f[:], 1.0)
        ones_row = wpool.tile([1, P], F32R)          # K=1 broadcast lhsT
        nc.scalar.copy(ones_row[:], ones_f[0:1, :])
        ones_col = wpool.tile([P, 1], F32R)          # partition-sum lhsT
        nc.scalar.copy(ones_col[:], ones_f[:, 0:1])
        ident_f = wpool.tile([P, P], F32)
        masks.make_identity(nc, ident_f[:])
        ident = wpool.tile([P, P], F32R)
        nc.scalar.copy(ident[:], ident_f[:])
        ident16 = wpool.tile([P, P], BF16)
        nc.scalar.copy(ident16[:], ident_f[:])
        eps_col = wpool.tile([P, 1], F32)
        nc.vector.memset(eps_col[:], 1e-5)
        # selector weights: sel_cols[k, g] = 1 if k == 16 + g else 0.
        # Broadcast along the free (M) dim as matmul lhsT to replicate row
        # 16+g of the 48-row dbl tile across all 128 output partitions
        # (PE operands must start at partition 0/32/64).
        sel_f = wpool.tile([R + 2 * N, 2 * N], F32)
        nc.gpsimd.memset(sel_f[:], 0.0)
        nc.gpsimd.affine_select(
            out=sel_f[:], in_=sel_f[:],
            compare_op=OP.not_equal, fill=1.0,
            base=-R, channel_multiplier=1, pattern=[[-1, 2 * N]])
        sel_cols = wpool.tile([R + 2 * N, 2 * N], F32R)
        nc.scalar.copy(sel_cols[:], sel_f[:])

        def sel_lhsT(g):
            return r32(sel_cols[:, g:g + 1].to_broadcast((R + 2 * N, P)))

        def wload(name, shape, src_ap, dtype=F32):
            t = wpool.tile(list(shape), dtype, name=name, tag=name)
            nc.sync.dma_start(out=t[:], in_=src_ap)
            return t

        # ---- persistent weights in SBUF
        wtok_sb = []
        off = 0
        for i, k in enumerate(TOK_K):
            wtok_sb.append(wload(f"wtok{i}", [k, D], dr["wtok"][off:off + k, :],
                                 F32R))
            off += k

        inw_sb = [[wload(f"inw{li}_{kt}", [P, 2 * DI],
                         dr["inw"][li, kt * P:(kt + 1) * P, :], F32R)
                   for kt in range(NKT_D)] for li in range(NL)]
        dtw_sb = [wload(f"dtw{li}", [R, DI], dr["dtw"][li], F32R)
                  for li in range(NL)]

        def dslice(li, dt_i):
            return (li, slice(dt_i * P, (dt_i + 1) * P))

        xpw_sb, outw_sb, convw_sb = [], [], []
        convb_sb, dtb_sb, amat_sb, dsk_sb = [], [], [], []
        for li in range(NL):
            xpw_sb.append([wload(f"xpw{li}_{i}", [P, R + 2 * N],
                                 dr["xpw"][dslice(li, i)], F32R)
                           for i in range(NDT)])
            outw_sb.append([wload(f"outw{li}_{i}", [P, D],
                                  dr["outw"][dslice(li, i)], F32R)
                            for i in range(NDT)])
            convw_sb.append([wload(f"convw{li}_{i}", [P, KC],
                                   dr["convw"][dslice(li, i)]) for i in range(NDT)])
            convb_sb.append([wload(f"convb{li}_{i}", [P, 1],
                                   dr["convb"][li, dt_i * P:(dt_i + 1) * P, None])
                             for dt_i in range(NDT)])
            dtb_sb.append([wload(f"dtb{li}_{i}", [P, 1],
                                 dr["dtb"][li, i * P:(i + 1) * P, None])
                           for i in range(NDT)])
            amat_sb.append([wload(f"amat{li}_{i}", [P, N],
                                  dr["amat"][dslice(li, i)]) for i in range(NDT)])
            dsk_sb.append([wload(f"dsk{li}_{i}", [P, 1],
                                 dr["dsk"][li, i * P:(i + 1) * P, None])
                           for i in range(NDT)])

        headw_sb = [wload(f"headw{kt}", [P, 3],
                          dr["headw"][kt * P:(kt + 1) * P, :], F32R)
                    for kt in range(NKT_D)]
        headb_sb = wload("headb", [3, 1], dr["headb"][:, None])
        tokg_sb = [wload(f"tokg{i}", [P, 1], dr["tokg"][i * P:(i + 1) * P, None])
                   for i in range(NKT_D)]
        tokb_sb = [wload(f"tokb{i}", [P, 1], dr["tokb"][i * P:(i + 1) * P, None])
                   for i in range(NKT_D)]
        normg_sb = [wload(f"normg{i}", [P, 1], dr["normg"][i * P:(i + 1) * P, None])
                    for i in range(NKT_D)]
        normb_sb = [wload(f"normb{i}", [P, 1], dr["normb"][i * P:(i + 1) * P, None])
                    for i in range(NKT_D)]

        # ---- persistent activations [channels, T]
        feats = [apool.tile([P, T], F32R, name=f"feats{i}", tag=f"feats{i}")
                 for i in range(NKT_D)]

        def layernorm(xs, dst, g_cols, b_cols, scratch_pool):
            """LN over the channel dim (on partitions, NKT_D k-tiles).

            xs: SBUF [P,T] tiles; dst may alias xs.
            """
            with tc.tile_pool(name="ln_ps", bufs=1, space="PSUM") as ps:
                stat_s = ps.tile([1, T], F32, name="stat_s", tag="stat_s")
                stat_q = ps.tile([1, T], F32, name="stat_q", tag="stat_q")
                for kt in range(NKT_D):
                    x2 = scratch_pool.tile([P, T], F32R, name="ln_x2", tag="ln_x2")
                    nc.scalar.square(x2[:], xs[kt][:])
                    for nh in range(NNH):
                        cs = slice(nh * 512, (nh + 1) * 512)
                        nc.tensor.matmul(stat_s[:, cs], r32(ones_col[:]),
                                         r32(xs[kt][:, cs]),
                                         start=(kt == 0), stop=(kt == NKT_D - 1))
                        nc.tensor.matmul(stat_q[:, cs], r32(ones_col[:]),
                                         r32(x2[:, cs]),
                                         start=(kt == 0), stop=(kt == NKT_D - 1))
                mean = scratch_pool.tile([1, T], F32R, name="ln_mean", tag="ln_mean")
                nc.vector.tensor_scalar_mul(mean[:], stat_s[:], 1.0 / D)
                msq = scratch_pool.tile([1, T], F32, name="ln_msq", tag="ln_msq")
                nc.vector.tensor_mul(msq[:], mean[:], mean[:])
                # var (into msq), then sd = sqrt(var+eps), rstd = 1/sd
                nc.vector.scalar_tensor_tensor(
                    out=msq[:], in0=stat_q[:], scalar=1.0 / D, in1=msq[:],
                    op0=OP.mult, op1=OP.subtract)
                nc.scalar.activation(msq[:], msq[:], AF.Sqrt,
                                     bias=eps_col[0:1, :])
                rstd = scratch_pool.tile([1, T], F32R, name="ln_rstd",
                                         tag="ln_rstd")
                with nc.allow_low_precision(reason="f32r rounding for PE bcast"):
                    nc.vector.reciprocal(rstd[:], msq[:])
                mean_bc = ps.tile([P, T], F32, name="ln_meanbc", tag="ln_meanbc")
                rstd_bc = ps.tile([P, T], F32, name="ln_rstdbc", tag="ln_rstdbc")
                for nh in range(NNH):
                    cs = slice(nh * 512, (nh + 1) * 512)
                    nc.tensor.matmul(mean_bc[:, cs], r32(ones_row[:]),
                                     r32(mean[:, cs]), start=True, stop=True)
                    nc.tensor.matmul(rstd_bc[:, cs], r32(ones_row[:]),
                                     r32(rstd[:, cs]), start=True, stop=True)
                for kt in range(NKT_D):
                    t1 = scratch_pool.tile([P, T], F32, name="ln_t1", tag="ln_t1")
                    nc.vector.tensor_sub(t1[:], xs[kt][:], mean_bc[:])
                    nc.vector.tensor_mul(t1[:], t1[:], rstd_bc[:])
                    nc.vector.scalar_tensor_tensor(
                        out=dst[kt][:], in0=t1[:], scalar=g_cols[kt][:],
                        in1=b_cols[kt][:].to_broadcast((P, T)),
                        op0=OP.mult, op1=OP.add)

        # ================= tokenizer =================
        with tc.tile_pool(name="tokpool", bufs=1) as tokpool:
            x0_bc = tokpool.tile([P, T], F32R)
            nc.sync.dma_start(out=x0_bc[:], in_=x5[0:1, :].to_broadcast((P, T)))
            x2_bc = tokpool.tile([64, T], F32R)
            nc.sync.dma_start(out=x2_bc[:], in_=x5[2:3, :].to_broadcast((64, T)))

            thr1 = tokpool.tile([P, 1], F32)
            nc.gpsimd.iota(thr1[:], pattern=[[0, 1]], base=1, channel_multiplier=1,
                           allow_small_or_imprecise_dtypes=True)
            thr129 = tokpool.tile([P, 1], F32)
            nc.gpsimd.iota(thr129[:], pattern=[[0, 1]], base=129,
                           channel_multiplier=1,
                           allow_small_or_imprecise_dtypes=True)

            cmp0 = tokpool.tile([P, T], F32R)
            nc.vector.tensor_tensor(cmp0[:], x0_bc[:],
                                    thr1[:].to_broadcast((P, T)), OP.is_ge)
            cmp1 = tokpool.tile([127, T], F32R)
            nc.vector.tensor_tensor(cmp1[:], x0_bc[0:127, :],
                                    thr129[0:127, :].to_broadcast((127, T)),
                                    OP.is_ge)
            cmp2f = tokpool.tile([68, T], F32)
            nc.vector.memset(cmp2f[:], 0.0)
            nc.vector.tensor_tensor(cmp2f[0:63, :], x2_bc[0:63, :],
                                    thr1[0:63, :].to_broadcast((63, T)),
                                    OP.is_ge)
            # rows 64-67 start as 1.0 (row 67 stays: the const feature);
            # rows 64-66 are then overwritten by dir/len/iat below.
            nc.vector.memset(cmp2f[64:68, :], 1.0)
            # dir flag: load x4 into its row, then compare >= 1 in place
            # (row 64 so the compute op starts at a legal partition)
            nc.gpsimd.dma_start(out=cmp2f[64:65, :], in_=x5[4:5, :].bitcast(F32))
            nc.gpsimd.tensor_scalar(cmp2f[64:65, :], cmp2f[64:65, :], 1.0, None,
                                    OP.is_ge)
            nc.gpsimd.dma_start(out=cmp2f[65:66, :], in_=x5[1:2, :].bitcast(F32))
            nc.gpsimd.dma_start(out=cmp2f[66:67, :], in_=x5[3:4, :].bitcast(F32))
            cmp2 = tokpool.tile([68, T], F32R)
            nc.scalar.copy(cmp2[:], cmp2f[:])

            cmps = [cmp0, cmp1, cmp2]
            xs = [tokpool.tile([P, T], F32R, name=f"tok_x{kt}", tag=f"tok_x{kt}")
                  for kt in range(NKT_D)]
            with tc.tile_pool(name="tok_ps", bufs=1, space="PSUM") as tokps:
                for mt in range(NKT_D):
                    fpre = tokps.tile([P, T], F32, name="fpre", tag="fpre")
                    for nh in range(NNH):
                        cs = slice(nh * 512, (nh + 1) * 512)
                        for ki in range(3):
                            nc.tensor.matmul(
                                fpre[:, cs],
                                r32(wtok_sb[ki][:, mt * P:(mt + 1) * P]),
                                r32(cmps[ki][:, cs]),
                                start=(ki == 0), stop=(ki == 2))
                    nc.scalar.copy(xs[mt][:], fpre[:])
            layernorm(xs, feats, tokg_sb, tokb_sb, tokpool)

        # ================= mamba layers =================
        lctx = ExitStack()
        lpool = lctx.enter_context(tc.tile_pool(name="lpool", bufs=1))
        u_t = [lpool.tile([P, T], F32R, name=f"u{i}", tag=f"u{i}")
               for i in range(NDT)]
        z_t = [lpool.tile([P, T], F32, name=f"z{i}", tag=f"z{i}")
               for i in range(NDT)]
        dt_t = [lpool.tile([P, T], F32, name=f"dt{i}", tag=f"dt{i}")
                for i in range(NDT)]
        dtu_t = [lpool.tile([P, T], F32R, name=f"dtu{i}", tag=f"dtu{i}")
                 for i in range(NDT)]
        dtub_t = [lpool.tile([P, T], BF16, name=f"dtub{i}", tag=f"dtub{i}")
                  for i in range(NDT)]
        for li in range(NL):
            # ---- in_proj: xz[o, t] = sum_k inw[k, o] * feats[k, t]
            with tc.tile_pool(name=f"ps_in{li}", bufs=2, space="PSUM") as psin:
                for mt in range(2 * DI // P):  # 8 output tiles: 0-3 u, 4-7 z
                    xz = psin.tile([P, T], F32, name="xz", tag="xz")
                    for nh in range(NNH):
                        cs = slice(nh * 512, (nh + 1) * 512)
                        for kt in range(NKT_D):
                            nc.tensor.matmul(
                                xz[:, cs],
                                r32(inw_sb[li][kt][:, mt * P:(mt + 1) * P]),
                                r32(feats[kt][:, cs]),
                                start=(kt == 0), stop=(kt == NKT_D - 1))
                    if mt < NDT:
                        nc.scalar.copy(u_t[mt][:], xz[:])
                    else:
                        # z gate: silu(z) = z * sigmoid(z)
                        j = mt - NDT
                        nc.scalar.copy(z_t[j][:], xz[:])
                        sg = spool.tile([P, T], F32, name="sg", tag="sg")
                        nc.scalar.activation(sg[:], xz[:], AF.Sigmoid)
                        nc.vector.tensor_mul(z_t[j][:], z_t[j][:], sg[:])

            # ---- causal depthwise conv + bias + silu (per batch row)
            for dt_i in range(NDT):
                acc = spool.tile([P, T], F32, name="cacc", tag="cacc")
                w = convw_sb[li][dt_i]
                for b in range(BL):
                    lo = b * L
                    # first tap also folds in the conv bias
                    nc.vector.tensor_scalar(
                        acc[:, lo:lo + L], u_t[dt_i][:, lo:lo + L], w[:, 3:4],
                        convb_sb[li][dt_i][:], OP.mult, OP.add)
                    for s in (1, 2, 3):  # u[l-s] * w[3-s]
                        nc.vector.scalar_tensor_tensor(
                            out=acc[:, lo + s:lo + L],
                            in0=u_t[dt_i][:, lo:lo + L - s],
                            scalar=w[:, 3 - s:4 - s],
                            in1=acc[:, lo + s:lo + L],
                            op0=OP.mult, op1=OP.add)
                # silu(acc) = acc * sigmoid(acc), overwrites u (dead)
                sg = spool.tile([P, T], F32, name="sg2", tag="sg")
                nc.scalar.activation(sg[:], acc[:], AF.Sigmoid)
                nc.vector.tensor_mul(u_t[dt_i][:], acc[:], sg[:])

            # ---- dbl = u @ xp_w.T -> [48, T] (dt_raw | B | C)
            dblc = spool.tile([R + 2 * N, T], F32R, name="dblc", tag="dblc")
            with tc.tile_pool(name=f"ps_dbl{li}", bufs=1, space="PSUM") as psdbl:
                dbl = psdbl.tile([R + 2 * N, T], F32)
                for nh in range(NNH):
                    cs = slice(nh * 512, (nh + 1) * 512)
                    for kt in range(NDT):
                        nc.tensor.matmul(
                            dbl[:, cs], r32(xpw_sb[li][kt][:]),
                            r32(u_t[kt][:, cs]),
                            start=(kt == 0), stop=(kt == NDT - 1))
                nc.scalar.copy(dblc[:], dbl[:])

            # ---- dt = softplus(dtraw @ dt_w.T + dt_b); dtu = dt * u
            with tc.tile_pool(name=f"ps_dt{li}", bufs=2, space="PSUM") as psdt:
                for dt_i in range(NDT):
                    dtp = psdt.tile([P, T], F32, name="dtp", tag="dtp")
                    for nh in range(NNH):
                        cs = slice(nh * 512, (nh + 1) * 512)
                        nc.tensor.matmul(
                            dtp[:, cs],
                            r32(dtw_sb[li][:, dt_i * P:(dt_i + 1) * P]),
                            r32(dblc[0:R, cs]), start=True, stop=True)
                    # softplus(x+b) = ln(1 + e^(x+b)); the Softplus ACT
                    # table crashes walrus lower_act, Exp/Ln share one set.
                    # |x+b| < ~6 here so e^x cannot overflow.
                    nc.scalar.activation(dt_t[dt_i][:], dtp[:], AF.Exp,
                                         bias=dtb_sb[li][dt_i][:])
                    nc.scalar.activation(dt_t[dt_i][:], dt_t[dt_i][:], AF.Ln,
                                         bias=1.0)
                    nc.gpsimd.tensor_mul(dtub_t[dt_i][:], dt_t[dt_i][:],
                                         u_t[dt_i][:])

            # ---- selective scan over n-states x d-tiles
            # bf16 copy of dbl so B/C rows can be DMA-broadcast cheaply
            dblc16 = spool.tile([R + 2 * N, T], BF16, name="dblc16",
                                tag="dblc16")
            nc.scalar.copy(dblc16[:], dblc[:])
            # bounce B/C rows through DRAM: SBUF sources cannot be
            # partition-broadcast by DMA, DRAM sources can
            bc_dram = dpool.tile([2 * N, T], BF16, name="bc_dram",
                                 tag="bc_dram")
            nc.sync.dma_start(out=bc_dram[:], in_=dblc16[R:R + 2 * N, :])
            with tc.tile_pool(name=f"ps_scan{li}", bufs=1,
                              space="PSUM") as pscan:
                ys = {}
                for dt_i in range(NDT):
                    ys[dt_i] = pscan.tile([P, T], F32, name=f"y{dt_i}",
                                          tag=f"y{dt_i}")
                for n in range(N):
                    b16 = spool.tile([P, T], BF16, name="b16", tag="b16")
                    nc.sync.dma_start(
                        out=b16[:],
                        in_=bc_dram[n:n + 1, :].to_broadcast((P, T)))
                    c16 = spool.tile([P, T], BF16, name="c16", tag="c16")
                    nc.sync.dma_start(
                        out=c16[:],
                        in_=bc_dram[N + n:N + n + 1, :].to_broadcast((P, T)))
                    for dt_i in range(NDT):
                        dA = spool.tile([P, T], BF16, name="dA", tag="dA")
                        nc.scalar.activation(
                            dA[:], dt_t[dt_i][:], AF.Exp,
                            scale=amat_sb[li][dt_i][:, n:n + 1])
                        dBu = spool.tile([P, T], BF16, name="dBu", tag="dBu")
                        nc.gpsimd.tensor_mul(dBu[:], dtub_t[dt_i][:], b16[:])
                        h = spool.tile([P, T], BF16, name="h", tag="h")
                        for b in range(BL):
                            lo = b * L
                            nc.vector.tensor_tensor_scan(
                                h[:, lo:lo + L], dA[:, lo:lo + L],
                                dBu[:, lo:lo + L], 0.0, OP.mult, OP.add)
                        # prod = h * C  (reuses dBu buffer, now dead)
                        nc.vector.tensor_mul(dBu[:], h[:], c16[:])
                        for nh in range(NNH):
                            cs = slice(nh * 512, (nh + 1) * 512)
                            nc.tensor.matmul(
                                ys[dt_i][:, cs], ident16[:],
                                dBu[:, cs],
                                start=(n == 0), stop=(n == N - 1))
                for dt_i in range(NDT):
                    # y_full = y + u * D_skip (into dtu, which is dead),
                    # then gate with silu(z)
                    nc.vector.scalar_tensor_tensor(
                        out=dtu_t[dt_i][:], in0=u_t[dt_i][:],
                        scalar=dsk_sb[li][dt_i][:], in1=ys[dt_i][:],
                        op0=OP.mult, op1=OP.add)
                    nc.gpsimd.tensor_mul(dtu_t[dt_i][:], dtu_t[dt_i][:],
                                         z_t[dt_i][:])

            # ---- out_proj: feats[o, t] = sum_d outw[d, o] * y_gated[d, t]
            with tc.tile_pool(name=f"ps_out{li}", bufs=2, space="PSUM") as psout:
                for mt in range(NKT_D):
                    op_ps = psout.tile([P, T], F32, name="op_ps", tag="op_ps")
                    for nh in range(NNH):
                        cs = slice(nh * 512, (nh + 1) * 512)
                        for kt in range(NDT):
                            nc.tensor.matmul(
                                op_ps[:, cs],
                                r32(outw_sb[li][kt][:, mt * P:(mt + 1) * P]),
                                r32(dtu_t[kt][:, cs]),
                                start=(kt == 0), stop=(kt == NDT - 1))
                    nc.scalar.copy(feats[mt][:], op_ps[:])

        # ================= final LN + head =================
        lctx.close()
        with tc.tile_pool(name="headpool", bufs=1) as hpool:
            layernorm(feats, feats, normg_sb, normb_sb, hpool)
            with tc.tile_pool(name="head_ps", bufs=1, space="PSUM") as hps:
                hd = hps.tile([3, T], F32)
                for nh in range(NNH):
                    cs = slice(nh * 512, (nh + 1) * 512)
                    for kt in range(NKT_D):
                        nc.tensor.matmul(
                            hd[:, cs], r32(headw_sb[kt][:]),
                            r32(feats[kt][:, cs]),
                            start=(kt == 0), stop=(kt == NKT_D - 1))
                # compute identity(+bias) and sigmoid(+bias) on the full
                # [3,T] tile (partition starts must be 0/32/64/96), then DMA
                # logits rows from one and the halt row from the other.
                out3_sb = hpool.tile([3, T], F32)
                nc.scalar.activation(out3_sb[:], hd[:], AF.Identity,
                                     bias=headb_sb[:])
                sig_sb = hpool.tile([3, T], F32)
                nc.scalar.activation(sig_sb[:], hd[:], AF.Sigmoid,
                                     bias=headb_sb[:])
                nc.sync.dma_start(out=out3_dram[0:2, :], in_=out3_sb[0:2, :])
                nc.sync.dma_start(out=out3_dram[2:3, :], in_=sig_sb[2:3, :])

    # bacc lowering: register allocation, nop fusion, multi-wait split
    nc.compile()
    return nc


_CACHED = {}


def get_program():
    if "nc" not in _CACHED:
        _CACHED["nc"] = build_program()
    return _CACHED["nc"]


def make_in_maps(inputs):
    prep = host_prep(inputs)
    x = np.asarray(inputs["x"], np.float32)
    in_maps = []
    for c in range(NCORES):
        shard = x[c * BL:(c + 1) * BL].reshape(T, 5).T
        m = {"x5": np.ascontiguousarray(shard)}
        m.update(prep)
        in_maps.append(m)
    return in_maps


def postprocess(results):
    logits = np.empty((B, L, 2), np.float32)
    halt = np.empty((B, L), np.float32)
    for c, res in enumerate(results):
        arr = np.asarray(res["out3"]).reshape(3, BL, L)
        logits[c * BL:(c + 1) * BL] = np.moveaxis(arr[0:2], 0, -1)
        halt[c * BL:(c + 1) * BL] = arr[2]
    return logits, halt


def kernel(**inputs):
    nc = get_program()
    in_maps = make_in_maps(inputs)
    res = run_bass_kernel_spmd(nc, in_maps, core_ids=list(range(NCORES)))
    return postprocess(res.results)


# revision 38
# speedup vs baseline: 1.2482x; 1.2482x over previous
"""Trainium2 Bass kernel for nn_DynamicEarlyExitStudent.

3-layer Mamba-style selective-SSM student network:
  tokenize (embed lookups + linear fusion + LN) -> 3x mamba -> LN -> heads.

Sharding: data-parallel over batch B=16 across 8 cores (2 rows/core),
params replicated, no collectives. Each core computes its rows end-to-end.

Device layout: activations are kept transposed [channels, tokens] with
channels on SBUF partitions; tokens t = b*512 + l (2 local batch rows).
The L-sequential scan runs on the Vector engine via tensor_tensor_scan
(state = dA*state + dBu along the free dim), vectorized over 128 channels
per instruction, one instruction per (state-index n, d-tile, batch row).
Embedding lookups are folded into threshold-comparison features times a
host-precomposed matrix; the n-reduction of h*C runs on the TensorEngine
as identity-matmul PSUM accumulation. Matmuls use float32r (full-rate).
"""

import sys

for _p in ("/opt/trn_rl_repo",):
    if _p not in sys.path:
        sys.path.insert(0, _p)

from contextlib import ExitStack

import numpy as np

import concourse.bacc as bacc
import concourse.bass as bass
import concourse.tile as tile
from concourse import masks, mybir
from concourse.bass_utils import run_bass_kernel_spmd

# Model dims
D, DI, N, R, KC, NL = 256, 512, 16, 16, 4, 3
B, L = 16, 512
NCORES = 8
BL = B // NCORES          # local batch rows per core
T = BL * L                # local tokens
P = 128                   # SBUF partitions
F32 = mybir.dt.float32
F32R = mybir.dt.float32r
BF16 = mybir.dt.bfloat16
AF = mybir.ActivationFunctionType
OP = mybir.AluOpType

NKT_D = D // P            # 2 k-tiles over D
NDT = DI // P             # 4 d-tiles over DI
NNH = T // 512            # matmul N-halves (fp32 moving max 512)

# tokenizer comparison-feature k-tiles (rows per tile). Tile 2 layout:
# [0:63]=flag thresholds 1..63, 63=unused(zero), 64=dir, 65=len, 66=iat,
# 67=const  (engine writes may only start at partitions 0/32/64/96).
TOK_K = (128, 127, 68)


def r32(ap):
    return ap.bitcast(F32R)


def host_prep(inputs):
    """Precompute combined weight matrices on host (small, param-only)."""
    f = lambda k: np.asarray(inputs[k], np.float32)
    emb_proto, emb_flags, emb_dir = f("emb_proto"), f("emb_flags"), f("emb_dir")
    fus_w, fus_b = f("fus_w"), f("fus_b")
    len_w, len_b, iat_w, iat_b = f("len_w"), f("len_b"), f("iat_w"), f("iat_b")
    Wp, Wf, Wd = fus_w[:, 0:32], fus_w[:, 32:64], fus_w[:, 64:72]
    Wl, Wi = fus_w[:, 72:104], fus_w[:, 104:136]

    # one-hot(trunc(x)) @ E == const_row + sum_v (x>=v) * (E[v]-E[v-1])
    dproto = (emb_proto[1:] - emb_proto[:-1]) @ Wp.T          # [255, D]
    dflags = (emb_flags[1:] - emb_flags[:-1]) @ Wf.T          # [63, D]
    ddir = (emb_dir[1] - emb_dir[0]) @ Wd.T                   # [D]
    row_len = len_w[:, 0] @ Wl.T                              # [D]
    row_iat = iat_w[:, 0] @ Wi.T                              # [D]
    row_const = (
        emb_proto[0] @ Wp.T + emb_flags[0] @ Wf.T + emb_dir[0] @ Wd.T
        + len_b @ Wl.T + iat_b @ Wi.T + fus_b
    )
    wtok = np.zeros((sum(TOK_K), D), np.float32)
    wtok[0:255] = dproto
    wtok[255:318] = dflags
    wtok[319] = ddir
    wtok[320] = row_len
    wtok[321] = row_iat
    wtok[322] = row_const

    prep = {
        "wtok": wtok,
        "inw": np.ascontiguousarray(f("in_w").transpose(0, 2, 1)),    # [NL,256,1024]
        "xpw": np.ascontiguousarray(f("xp_w").transpose(0, 2, 1)),    # [NL,512,48]
        "dtw": np.ascontiguousarray(f("dt_w").transpose(0, 2, 1)),    # [NL,16,512]
        "outw": np.ascontiguousarray(f("out_w").transpose(0, 2, 1)),  # [NL,512,256]
        "convw": np.ascontiguousarray(f("conv_w")[:, :, 0, :]),       # [NL,512,4]
        "convb": f("conv_b"),
        "dtb": f("dt_b"),
        "amat": np.ascontiguousarray(-np.exp(f("A_log"))),            # [NL,512,16]
        "dsk": f("D_skip"),
        "tokg": f("tok_g"), "tokb": f("tok_b"),
        "normg": f("norm_g"), "normb": f("norm_b"),
        "headw": np.ascontiguousarray(
            np.concatenate([f("cls_w"), f("halt_w")], 0).T),          # [256, 3]
        "headb": np.concatenate([f("cls_b"), f("halt_b")], 0),        # [3]
    }
    return prep


# (shape, is_matmul_operand) - matmul operands are float32r end to end
PARAM_SPECS = {
    "wtok": ((sum(TOK_K), D), True),
    "inw": ((NL, D, 2 * DI), True),
    "xpw": ((NL, DI, R + 2 * N), True),
    "dtw": ((NL, R, DI), True),
    "outw": ((NL, DI, D), True),
    "convw": ((NL, DI, KC), False),
    "convb": ((NL, DI), False),
    "dtb": ((NL, DI), False),
    "amat": ((NL, DI, N), False),
    "dsk": ((NL, DI), False),
    "tokg": ((D,), False), "tokb": ((D,), False),
    "normg": ((D,), False), "normb": ((D,), False),
    "headw": ((D, 3), True),
    "headb": ((3,), False),
}


def build_program():
    nc = bacc.Bacc("TRN2", target_bir_lowering=False, debug=False)

    x5 = nc.dram_tensor("x5", [5, T], F32R, kind="ExternalInput").ap()
    dr = {
        k: nc.dram_tensor(k, list(shp), F32R if isr else F32,
                          kind="ExternalInput").ap()
        for k, (shp, isr) in PARAM_SPECS.items()
    }
    out3_dram = nc.dram_tensor("out3", [3, T], F32, kind="ExternalOutput").ap()

    with tile.TileContext(nc) as tc, ExitStack() as ctx:
        wpool = ctx.enter_context(tc.tile_pool(name="wpool", bufs=1))
        dpool = ctx.enter_context(tc.tile_pool(name="dpool", bufs=2,
                                               space="DRAM"))
        apool = ctx.enter_context(tc.tile_pool(name="apool", bufs=1))
        spool = ctx.enter_context(tc.tile_pool(name="spool", bufs=3))

        # ---- constants (f32r tiles cannot be memset directly; build in
        # f32 and convert via ScalarE copy, which rounds to f32r on write)
        ones_f = wpool.tile([P, P], F32)
        nc.vector.memset(ones_f[:], 1.0)
        ones_row = wpool.tile([1, P], F32R)          # K=1 broadcast lhsT
        nc.scalar.copy(ones_row[:], ones_f[0:1, :])
        ones_col = wpool.tile([P, 1], F32R)          # partition-sum lhsT
        nc.scalar.copy(ones_col[:], ones_f[:, 0:1])
        ident_f = wpool.tile([P, P], F32)
        masks.make_identity(nc, ident_f[:])
        ident = wpool.tile([P, P], F32R)
        nc.scalar.copy(ident[:], ident_f[:])
        ident16 = wpool.tile([P, P], BF16)
        nc.scalar.copy(ident16[:], ident_f[:])
        eps_col = wpool.tile([P, 1], F32)
        nc.vector.memset(eps_col[:], 1e-5)
        # selector weights: sel_cols[k, g] = 1 if k == 16 + g else 0.
        # Broadcast along the free (M) dim as matmul lhsT to replicate row
        # 16+g of the 48-row dbl tile across all 128 output partitions
        # (PE operands must start at partition 0/32/64).
        sel_f = wpool.tile([R + 2 * N, 2 * N], F32)
        nc.gpsimd.memset(sel_f[:], 0.0)
        nc.gpsimd.affine_select(
            out=sel_f[:], in_=sel_f[:],
            compare_op=OP.not_equal, fill=1.0,
            base=-R, channel_multiplier=1, pattern=[[-1, 2 * N]])
        sel_cols = wpool.tile([R + 2 * N, 2 * N], F32R)
        nc.scalar.copy(sel_cols[:], sel_f[:])

        def sel_lhsT(g):
            return r32(sel_cols[:, g:g + 1].to_broadcast((R + 2 * N, P)))

        def wload(name, shape, src_ap, dtype=F32):
            t = wpool.tile(list(shape), dtype, name=name, tag=name)
            nc.sync.dma_start(out=t[:], in_=src_ap)
            return t

        # ---- persistent weights in SBUF
        wtok_sb = []
        off = 0
        for i, k in enumerate(TOK_K):
            wtok_sb.append(wload(f"wtok{i}", [k, D], dr["wtok"][off:off + k, :],
                                 F32R))
            off += k

        inw_sb = [[wload(f"inw{li}_{kt}", [P, 2 * DI],
                         dr["inw"][li, kt * P:(kt + 1) * P, :], F32R)
                   for kt in range(NKT_D)] for li in range(NL)]
        dtw_sb = [wload(f"dtw{li}", [R, DI], dr["dtw"][li], F32R)
                  for li in range(NL)]

        def dslice(li, dt_i):
            return (li, slice(dt_i * P, (dt_i + 1) * P))

        xpw_sb, outw_sb, convw_sb = [], [], []
        convb_sb, dtb_sb, amat_sb, dsk_sb = [], [], [], []
        for li in range(NL):
            xpw_sb.append([wload(f"xpw{li}_{i}", [P, R + 2 * N],
                                 dr["xpw"][dslice(li, i)], F32R)
                           for i in range(NDT)])
            outw_sb.append([wload(f"outw{li}_{i}", [P, D],
                                  dr["outw"][dslice(li, i)], F32R)
                            for i in range(NDT)])
            convw_sb.append([wload(f"convw{li}_{i}", [P, KC],
                                   dr["convw"][dslice(li, i)]) for i in range(NDT)])
            convb_sb.append([wload(f"convb{li}_{i}", [P, 1],
                                   dr["convb"][li, dt_i * P:(dt_i + 1) * P, None])
                             for dt_i in range(NDT)])
            dtb_sb.append([wload(f"dtb{li}_{i}", [P, 1],
                                 dr["dtb"][li, i * P:(i + 1) * P, None])
                           for i in range(NDT)])
            amat_sb.append([wload(f"amat{li}_{i}", [P, N],
                                  dr["amat"][dslice(li, i)]) for i in range(NDT)])
            dsk_sb.append([wload(f"dsk{li}_{i}", [P, 1],
                                 dr["dsk"][li, i * P:(i + 1) * P, None])
                           for i in range(NDT)])

        headw_sb = [wload(f"headw{kt}", [P, 3],
                          dr["headw"][kt * P:(kt + 1) * P, :], F32R)
                    for kt in range(NKT_D)]
        headb_sb = wload("headb", [3, 1], dr["headb"][:, None])
        tokg_sb = [wload(f"tokg{i}", [P, 1], dr["tokg"][i * P:(i + 1) * P, None])
                   for i in range(NKT_D)]
        tokb_sb = [wload(f"tokb{i}", [P, 1], dr["tokb"][i * P:(i + 1) * P, None])
                   for i in range(NKT_D)]
        normg_sb = [wload(f"normg{i}", [P, 1], dr["normg"][i * P:(i + 1) * P, None])
                    for i in range(NKT_D)]
        normb_sb = [wload(f"normb{i}", [P, 1], dr["normb"][i * P:(i + 1) * P, None])
                    for i in range(NKT_D)]

        # ---- persistent activations [channels, T]
        feats = [apool.tile([P, T], F32R, name=f"feats{i}", tag=f"feats{i}")
                 for i in range(NKT_D)]

        def layernorm(xs, dst, g_cols, b_cols, scratch_pool):
            """LN over the channel dim (on partitions, NKT_D k-tiles).

            xs: SBUF [P,T] tiles; dst may alias xs.
            """
            with tc.tile_pool(name="ln_ps", bufs=1, space="PSUM") as ps:
                stat_s = ps.tile([1, T], F32, name="stat_s", tag="stat_s")
                stat_q = ps.tile([1, T], F32, name="stat_q", tag="stat_q")
                for kt in range(NKT_D):
                    x2 = scratch_pool.tile([P, T], F32R, name="ln_x2", tag="ln_x2")
                    nc.scalar.square(x2[:], xs[kt][:])
                    for nh in range(NNH):
                        cs = slice(nh * 512, (nh + 1) * 512)
                        nc.tensor.matmul(stat_s[:, cs], r32(ones_col[:]),
                                         r32(xs[kt][:, cs]),
                                         start=(kt == 0), stop=(kt == NKT_D - 1))
                        nc.tensor.matmul(stat_q[:, cs], r32(ones_col[:]),
                                         r32(x2[:, cs]),
                                         start=(kt == 0), stop=(kt == NKT_D - 1))
                mean = scratch_pool.tile([1, T], F32R, name="ln_mean", tag="ln_mean")
                nc.vector.tensor_scalar_mul(mean[:], stat_s[:], 1.0 / D)
                msq = scratch_pool.tile([1, T], F32, name="ln_msq", tag="ln_msq")
                nc.vector.tensor_mul(msq[:], mean[:], mean[:])
                # var (into msq), then sd = sqrt(var+eps), rstd = 1/sd
                nc.vector.scalar_tensor_tensor(
                    out=msq[:], in0=stat_q[:], scalar=1.0 / D, in1=msq[:],
                    op0=OP.mult, op1=OP.subtract)
                nc.scalar.activation(msq[:], msq[:], AF.Sqrt,
                                     bias=eps_col[0:1, :])
                rstd = scratch_pool.tile([1, T], F32R, name="ln_rstd",
                                         tag="ln_rstd")
                with nc.allow_low_precision(reason="f32r rounding for PE bcast"):
                    nc.vector.reciprocal(rstd[:], msq[:])
                mean_bc = ps.tile([P, T], F32, name="ln_meanbc", tag="ln_meanbc")
                rstd_bc = ps.tile([P, T], F32, name="ln_rstdbc", tag="ln_rstdbc")
                for nh in range(NNH):
                    cs = slice(nh * 512, (nh + 1) * 512)
                    nc.tensor.matmul(mean_bc[:, cs], r32(ones_row[:]),
                                     r32(mean[:, cs]), start=True, stop=True)
                    nc.tensor.matmul(rstd_bc[:, cs], r32(ones_row[:]),
                                     r32(rstd[:, cs]), start=True, stop=True)
                for kt in range(NKT_D):
                    t1 = scratch_pool.tile([P, T], F32, name="ln_t1", tag="ln_t1")
                    nc.vector.tensor_sub(t1[:], xs[kt][:], mean_bc[:])
                    nc.vector.tensor_mul(t1[:], t1[:], rstd_bc[:])
                    nc.vector.scalar_tensor_tensor(
                        out=dst[kt][:], in0=t1[:], scalar=g_cols[kt][:],
                        in1=b_cols[kt][:].to_broadcast((P, T)),
                        op0=OP.mult, op1=OP.add)

        # ================= tokenizer =================
        with tc.tile_pool(name="tokpool", bufs=1) as tokpool:
            x0_bc = tokpool.tile([P, T], F32R)
            nc.sync.dma_start(out=x0_bc[:], in_=x5[0:1, :].to_broadcast((P, T)))
            x2_bc = tokpool.tile([64, T], F32R)
            nc.sync.dma_start(out=x2_bc[:], in_=x5[2:3, :].to_broadcast((64, T)))

            thr1 = tokpool.tile([P, 1], F32)
            nc.gpsimd.iota(thr1[:], pattern=[[0, 1]], base=1, channel_multiplier=1,
                           allow_small_or_imprecise_dtypes=True)
            thr129 = tokpool.tile([P, 1], F32)
            nc.gpsimd.iota(thr129[:], pattern=[[0, 1]], base=129,
                           channel_multiplier=1,
                           allow_small_or_imprecise_dtypes=True)

            cmp0 = tokpool.tile([P, T], F32R)
            nc.vector.tensor_tensor(cmp0[:], x0_bc[:],
                                    thr1[:].to_broadcast((P, T)), OP.is_ge)
            cmp1 = tokpool.tile([127, T], F32R)
            nc.vector.tensor_tensor(cmp1[:], x0_bc[0:127, :],
                                    thr129[0:127, :].to_broadcast((127, T)),
                                    OP.is_ge)
            cmp2f = tokpool.tile([68, T], F32)
            nc.vector.memset(cmp2f[:], 0.0)
            nc.vector.tensor_tensor(cmp2f[0:63, :], x2_bc[0:63, :],
                                    thr1[0:63, :].to_broadcast((63, T)),
                                    OP.is_ge)
            # rows 64-67 start as 1.0 (row 67 stays: the const feature);
            # rows 64-66 are then overwritten by dir/len/iat below.
            nc.vector.memset(cmp2f[64:68, :], 1.0)
            # dir flag: load x4 into its row, then compare >= 1 in place
            # (row 64 so the compute op starts at a legal partition)
            nc.gpsimd.dma_start(out=cmp2f[64:65, :], in_=x5[4:5, :].bitcast(F32))
            nc.gpsimd.tensor_scalar(cmp2f[64:65, :], cmp2f[64:65, :], 1.0, None,
                                    OP.is_ge)
            nc.gpsimd.dma_start(out=cmp2f[65:66, :], in_=x5[1:2, :].bitcast(F32))
            nc.gpsimd.dma_start(out=cmp2f[66:67, :], in_=x5[3:4, :].bitcast(F32))
            cmp2 = tokpool.tile([68, T], F32R)
            nc.scalar.copy(cmp2[:], cmp2f[:])

            cmps = [cmp0, cmp1, cmp2]
            xs = [tokpool.tile([P, T], F32R, name=f"tok_x{kt}", tag=f"tok_x{kt}")
                  for kt in range(NKT_D)]
            with tc.tile_pool(name="tok_ps", bufs=1, space="PSUM") as tokps:
                for mt in range(NKT_D):
                    fpre = tokps.tile([P, T], F32, name="fpre", tag="fpre")
                    for nh in range(NNH):
                        cs = slice(nh * 512, (nh + 1) * 512)
                        for ki in range(3):
                            nc.tensor.matmul(
                                fpre[:, cs],
                                r32(wtok_sb[ki][:, mt * P:(mt + 1) * P]),
                                r32(cmps[ki][:, cs]),
                                start=(ki == 0), stop=(ki == 2))
                    nc.scalar.copy(xs[mt][:], fpre[:])
            layernorm(xs, feats, tokg_sb, tokb_sb, tokpool)

        # ================= mamba layers =================
        lctx = ExitStack()
        lpool = lctx.enter_context(tc.tile_pool(name="lpool", bufs=1))
        u_t = [lpool.tile([P, T], F32R, name=f"u{i}", tag=f"u{i}")
               for i in range(NDT)]
        z_t = [lpool.tile([P, T], F32, name=f"z{i}", tag=f"z{i}")
               for i in range(NDT)]
        dt_t = [lpool.tile([P, T], F32, name=f"dt{i}", tag=f"dt{i}")
                for i in range(NDT)]
        dtu_t = [lpool.tile([P, T], F32R, name=f"dtu{i}", tag=f"dtu{i}")
                 for i in range(NDT)]
        dtub_t = [lpool.tile([P, T], BF16, name=f"dtub{i}", tag=f"dtub{i}")
                  for i in range(NDT)]
        for li in range(NL):
            # ---- in_proj: xz[o, t] = sum_k inw[k, o] * feats[k, t]
            with tc.tile_pool(name=f"ps_in{li}", bufs=2, space="PSUM") as psin:
                for mt in range(2 * DI // P):  # 8 output tiles: 0-3 u, 4-7 z
                    xz = psin.tile([P, T], F32, name="xz", tag="xz")
                    for nh in range(NNH):
                        cs = slice(nh * 512, (nh + 1) * 512)
                        for kt in range(NKT_D):
                            nc.tensor.matmul(
                                xz[:, cs],
                                r32(inw_sb[li][kt][:, mt * P:(mt + 1) * P]),
                                r32(feats[kt][:, cs]),
                                start=(kt == 0), stop=(kt == NKT_D - 1))
                    if mt < NDT:
                        nc.scalar.copy(u_t[mt][:], xz[:])
                    else:
                        # z gate: silu(z) = z * sigmoid(z)
                        j = mt - NDT
                        nc.scalar.copy(z_t[j][:], xz[:])
                        sg = spool.tile([P, T], F32, name="sg", tag="sg")
                        nc.scalar.activation(sg[:], xz[:], AF.Sigmoid)
                        nc.vector.tensor_mul(z_t[j][:], z_t[j][:], sg[:])

            # ---- causal depthwise conv + bias + silu (per batch row)
            for dt_i in range(NDT):
                acc = spool.tile([P, T], F32, name="cacc", tag="cacc")
                w = convw_sb[li][dt_i]
                for b in range(BL):
                    lo = b * L
                    # first tap also folds in the conv bias
                    nc.vector.tensor_scalar(
                        acc[:, lo:lo + L], u_t[dt_i][:, lo:lo + L], w[:, 3:4],
                        convb_sb[li][dt_i][:], OP.mult, OP.add)
                    for s in (1, 2, 3):  # u[l-s] * w[3-s]
                        nc.vector.scalar_tensor_tensor(
                            out=acc[:, lo + s:lo + L],
                            in0=u_t[dt_i][:, lo:lo + L - s],
                            scalar=w[:, 3 - s:4 - s],
                            in1=acc[:, lo + s:lo + L],
                            op0=OP.mult, op1=OP.add)
                # silu(acc) = acc * sigmoid(acc), overwrites u (dead)
                sg = spool.tile([P, T], F32, name="sg2", tag="sg")
                nc.scalar.activation(sg[:], acc[:], AF.Sigmoid)
                nc.vector.tensor_mul(u_t[dt_i][:], acc[:], sg[:])

            # ---- dbl = u @ xp_w.T -> [48, T] (dt_raw | B | C)
            dblc = spool.tile([R + 2 * N, T], F32R, name="dblc", tag="dblc")
            with tc.tile_pool(name=f"ps_dbl{li}", bufs=1, space="PSUM") as psdbl:
                dbl = psdbl.tile([R + 2 * N, T], F32)
                for nh in range(NNH):
                    cs = slice(nh * 512, (nh + 1) * 512)
                    for kt in range(NDT):
                        nc.tensor.matmul(
                            dbl[:, cs], r32(xpw_sb[li][kt][:]),
                            r32(u_t[kt][:, cs]),
                            start=(kt == 0), stop=(kt == NDT - 1))
                nc.scalar.copy(dblc[:], dbl[:])

            # ---- dt = softplus(dtraw @ dt_w.T + dt_b); dtu = dt * u
            with tc.tile_pool(name=f"ps_dt{li}", bufs=2, space="PSUM") as psdt:
                for dt_i in range(NDT):
                    dtp = psdt.tile([P, T], F32, name="dtp", tag="dtp")
                    for nh in range(NNH):
                        cs = slice(nh * 512, (nh + 1) * 512)
                        nc.tensor.matmul(
                            dtp[:, cs],
                            r32(dtw_sb[li][:, dt_i * P:(dt_i + 1) * P]),
                            r32(dblc[0:R, cs]), start=True, stop=True)
                    # softplus(x+b) = ln(1 + e^(x+b)); the Softplus ACT
                    # table crashes walrus lower_act, Exp/Ln share one set.
                    # |x+b| < ~6 here so e^x cannot overflow.
                    nc.scalar.activation(dt_t[dt_i][:], dtp[:], AF.Exp,
                                         bias=dtb_sb[li][dt_i][:])
                    nc.scalar.activation(dt_t[dt_i][:], dt_t[dt_i][:], AF.Ln,
                                         bias=1.0)
                    nc.vector.tensor_mul(dtub_t[dt_i][:], dt_t[dt_i][:],
                                         u_t[dt_i][:])

            # ---- selective scan over n-states x d-tiles
            # bf16 copy of dbl so B/C rows can be DMA-broadcast cheaply
            dblc16 = spool.tile([R + 2 * N, T], BF16, name="dblc16",
                                tag="dblc16")
            nc.scalar.copy(dblc16[:], dblc[:])
            # bounce B/C rows through DRAM: SBUF sources cannot be
            # partition-broadcast by DMA, DRAM sources can
            bc_dram = dpool.tile([2 * N, T], BF16, name="bc_dram",
                                 tag="bc_dram")
            nc.sync.dma_start(out=bc_dram[:], in_=dblc16[R:R + 2 * N, :])
            with tc.tile_pool(name=f"ps_scan{li}", bufs=1,
                              space="PSUM") as pscan:
                ys = {}
                for dt_i in range(NDT):
                    ys[dt_i] = pscan.tile([P, T], F32, name=f"y{dt_i}",
                                          tag=f"y{dt_i}")
                for n in range(N):
                    b16 = spool.tile([P, T], BF16, name="b16", tag="b16")
                    nc.sync.dma_start(
                        out=b16[:],
                        in_=bc_dram[n:n + 1, :].to_broadcast((P, T)))
                    c16 = spool.tile([P, T], BF16, name="c16", tag="c16")
                    nc.sync.dma_start(
                        out=c16[:],
                        in_=bc_dram[N + n:N + n + 1, :].to_broadcast((P, T)))
                    for dt_i in range(NDT):
                        dA = spool.tile([P, T], BF16, name="dA", tag="dA")
                        nc.scalar.activation(
                            dA[:], dt_t[dt_i][:], AF.Exp,
                            scale=amat_sb[li][dt_i][:, n:n + 1])
                        dBu = spool.tile([P, T], BF16, name="dBu", tag="dBu")
                        nc.vector.tensor_mul(dBu[:], dtub_t[dt_i][:], b16[:])
                        h = spool.tile([P, T], BF16, name="h", tag="h")
                        for b in range(BL):
                            lo = b * L
                            nc.vector.tensor_tensor_scan(
                                h[:, lo:lo + L], dA[:, lo:lo + L],
                                dBu[:, lo:lo + L], 0.0, OP.mult, OP.add)
                        # prod = h * C  (reuses dBu buffer, now dead)
                        nc.vector.tensor_mul(dBu[:], h[:], c16[:])
                        for nh in range(NNH):
                            cs = slice(nh * 512, (nh + 1) * 512)
                            nc.tensor.matmul(
                                ys[dt_i][:, cs], ident16[:],
                                dBu[:, cs],
                                start=(n == 0), stop=(n == N - 1))
                for dt_i in range(NDT):
                    # y_full = y + u * D_skip (into dtu, which is dead),
                    # then gate with silu(z)
                    nc.vector.scalar_tensor_tensor(
                        out=dtu_t[dt_i][:], in0=u_t[dt_i][:],
                        scalar=dsk_sb[li][dt_i][:], in1=ys[dt_i][:],
                        op0=OP.mult, op1=OP.add)
                    nc.vector.tensor_mul(dtu_t[dt_i][:], dtu_t[dt_i][:],
                                         z_t[dt_i][:])

            # ---- out_proj: feats[o, t] = sum_d outw[d, o] * y_gated[d, t]
            with tc.tile_pool(name=f"ps_out{li}", bufs=2, space="PSUM") as psout:
                for mt in range(NKT_D):
                    op_ps = psout.tile([P, T], F32, name="op_ps", tag="op_ps")
                    for nh in range(NNH):
                        cs = slice(nh * 512, (nh + 1) * 512)
                        for kt in range(NDT):
                            nc.tensor.matmul(
                                op_ps[:, cs],
                                r32(outw_sb[li][kt][:, mt * P:(mt + 1) * P]),
                                r32(dtu_t[kt][:, cs]),
                                start=(kt == 0), stop=(kt == NDT - 1))
                    nc.scalar.copy(feats[mt][:], op_ps[:])

        # ================= final LN + head =================
        lctx.close()
        with tc.tile_pool(name="headpool", bufs=1) as hpool:
            layernorm(feats, feats, normg_sb, normb_sb, hpool)
            with tc.tile_pool(name="head_ps", bufs=1, space="PSUM") as hps:
                hd = hps.tile([3, T], F32)
                for nh in range(NNH):
                    cs = slice(nh * 512, (nh + 1) * 512)
                    for kt in range(NKT_D):
                        nc.tensor.matmul(
                            hd[:, cs], r32(headw_sb[kt][:]),
                            r32(feats[kt][:, cs]),
                            start=(kt == 0), stop=(kt == NKT_D - 1))
                # compute identity(+bias) and sigmoid(+bias) on the full
                # [3,T] tile (partition starts must be 0/32/64/96), then DMA
                # logits rows from one and the halt row from the other.
                out3_sb = hpool.tile([3, T], F32)
                nc.scalar.activation(out3_sb[:], hd[:], AF.Identity,
                                     bias=headb_sb[:])
                sig_sb = hpool.tile([3, T], F32)
                nc.scalar.activation(sig_sb[:], hd[:], AF.Sigmoid,
                                     bias=headb_sb[:])
                nc.sync.dma_start(out=out3_dram[0:2, :], in_=out3_sb[0:2, :])
                nc.sync.dma_start(out=out3_dram[2:3, :], in_=sig_sb[2:3, :])

    # bacc lowering: register allocation, nop fusion, multi-wait split
    nc.compile()
    return nc


_CACHED = {}


def get_program():
    if "nc" not in _CACHED:
        _CACHED["nc"] = build_program()
    return _CACHED["nc"]


def make_in_maps(inputs):
    prep = host_prep(inputs)
    x = np.asarray(inputs["x"], np.float32)
    in_maps = []
    for c in range(NCORES):
        shard = x[c * BL:(c + 1) * BL].reshape(T, 5).T
        m = {"x5": np.ascontiguousarray(shard)}
        m.update(prep)
        in_maps.append(m)
    return in_maps


def postprocess(results):
    logits = np.empty((B, L, 2), np.float32)
    halt = np.empty((B, L), np.float32)
    for c, res in enumerate(results):
        arr = np.asarray(res["out3"]).reshape(3, BL, L)
        logits[c * BL:(c + 1) * BL] = np.moveaxis(arr[0:2], 0, -1)
        halt[c * BL:(c + 1) * BL] = arr[2]
    return logits, halt


def kernel(**inputs):
    nc = get_program()
    in_maps = make_in_maps(inputs)
    res = run_bass_kernel_spmd(nc, in_maps, core_ids=list(range(NCORES)))
    return postprocess(res.results)
